# revision 42
# baseline (speedup 1.0000x reference)
"""2-layer LSTM greedy decoder (H=4096, E=512, 15 steps) on 8 trn2 NeuronCores.

Tensor-parallel over the 4*H gate dimension, core c owning h indices
[c*512, (c+1)*512).  Weights quantized as e4m3 hi + e4m3 lo planes at scale
512 (8.6 effective bits; validated to preserve the greedy argmax trajectory
with rel err 4.3e-3).  The hi plane is SBUF-resident (loaded once); the lo
plane streams from HBM every step (1 B/weight).  All gate matvecs run as
fp8 DoubleRow matmuls (256-deep contraction, 2x column rate).

Activations split on device into e4m3 hi+lo pairs at scale 64 (M=2
stationary columns -> psum rows folded after).  Gates live in a 2D
[128,16] per-partition layout (psum column n = 4*l + gate) so bias add,
sigmoid/tanh, c/h updates and argmax all run partition-parallel.
"""

import numpy as np
import ml_dtypes

H = 4096
E = 512
T = 15
NCORES = 8
P = 128

Hc = H // NCORES          # 512 h elements per core
Gc = 4 * Hc               # 2048 gate columns per core
KH = H // 256             # 16 DoubleRow chunks per h vector
KX = E // 256             # 2 chunks for x
NCH0 = KX + KH            # 18 layer-0 chunks (x first, then h0)
NCH1 = 2 * KH             # 32 layer-1 chunks (h1 first, then h0)
NCH = NCH0 + NCH1         # 50 total
N_MV = 8                  # trailing hi chunks streamed (SBUF headroom)
NRES = NCH - N_MV
NSEG = 4                  # psum 512-column segments
CHB = 2 * Gc              # fp8 bytes per chunk per partition (4096)

SW = 512.0                # weight scale
SV = 64.0                 # activation scale
SINV = 1.0 / (SW * SV)    # 2^-15, applied inside sigmoid/tanh


def build_nc():
    import concourse.bass as bass
    import concourse.mybir as mybir
    import concourse.tile as tile
    from concourse import bacc, bass_isa

    dt = mybir.dt
    AF = mybir.ActivationFunctionType
    OP = mybir.AluOpType
    DR = mybir.MatmulPerfMode.DoubleRow
    f32, f16, f8, u32 = dt.float32, dt.float16, dt.float8e4, dt.uint32
    SIG, TANH, COPY = AF.Sigmoid, AF.Tanh, AF.Copy

    nc = bacc.Bacc("TRN2", target_bir_lowering=False, debug=False,
                   num_devices=NCORES)

    whir = nc.dram_tensor("whir", [P, NRES * CHB], f8, kind="ExternalInput")
    whim = nc.dram_tensor("whim", [P, N_MV * CHB], f8, kind="ExternalInput")
    wlod = nc.dram_tensor("wlod", [P, NCH * CHB], f8, kind="ExternalInput")
    b0d = nc.dram_tensor("b0d", [P, 16], f32, kind="ExternalInput")
    b1d = nc.dram_tensor("b1d", [P, 16], f32, kind="ExternalInput")
    embtd = nc.dram_tensor("embtd", [H * P, 4], f32, kind="ExternalInput")
    xt0d = nc.dram_tensor("xt0d", [P, 4], f32, kind="ExternalInput")
    iotd = nc.dram_tensor("iotd", [P, 1], u32, kind="ExternalInput")
    i32d = nc.dram_tensor("i32d", [P, 1], f32, kind="ExternalInput")
    outd = nc.dram_tensor("out", [T, Hc], f32, kind="ExternalOutput")

    with tile.TileContext(nc) as tc, \
            tc.tile_pool(name="wres", bufs=1) as wrp, \
            tc.tile_pool(name="wst", bufs=6) as wsp, \
            tc.tile_pool(name="act", bufs=1) as acp, \
            tc.tile_pool(name="gat", bufs=1) as gap, \
            tc.tile_pool(name="st", bufs=1) as stp, \
            tc.tile_pool(name="psum", bufs=2, space="PSUM") as psp, \
            tc.tile_pool(name="dram", bufs=2, space="DRAM") as drp:

        # ---- constants / persistent state ----
        # resident hi plane, split per layer so step-0 layer 0 starts early
        wres0 = wrp.tile([P, NCH0 * CHB], f8, tag="wres0", name="wres0")
        nc.sync.dma_start(out=wres0[:, :], in_=whir[:, :NCH0 * CHB])
        NR1 = NRES - NCH0
        wres1 = wrp.tile([P, NR1 * CHB], f8, tag="wres1", name="wres1")
        NLOAD = 4
        bnd = [NCH0 * CHB + round(i * NR1 / NLOAD) * CHB
               for i in range(NLOAD + 1)]
        for i in range(NLOAD):
            nc.sync.dma_start(
                out=wres1[:, bnd[i] - NCH0 * CHB:bnd[i + 1] - NCH0 * CHB],
                in_=whir[:, bnd[i]:bnd[i + 1]])

        def wres_ap(g):
            if g < NCH0:
                return wres0[:, g * CHB:(g + 1) * CHB]
            return wres1[:, (g - NCH0) * CHB:(g - NCH0 + 1) * CHB]

        bsb = {}
        for l, bd in ((0, b0d), (1, b1d)):
            bsb[l] = stp.tile([P, 16], f32, tag=f"b{l}sb", name=f"b{l}sb")
            nc.scalar.dma_start(out=bsb[l][:, :], in_=bd[:, :])
        iot = stp.tile([P, 1], u32, tag="iot", name="iot")
        nc.scalar.dma_start(out=iot[:, :], in_=iotd[:, :])
        p32 = stp.tile([P, 1], f32, tag="p32", name="p32")
        nc.scalar.dma_start(out=p32[:, :], in_=i32d[:, :])
        c_t = {}
        for l in (0, 1):
            c_t[l] = stp.tile([P, 4], f32, tag=f"c{l}", name=f"c{l}")
            nc.vector.memset(c_t[l][:, :], 0.0)

        # fp8 hi/lo splits of activations (slot0=hi, slot1=lo, 16B cells)
        hs8 = {0: acp.tile([P, KH * 2, 16], f8, tag="hs80", name="hs80"),
               1: acp.tile([P, KH * 2, 16], f8, tag="hs81", name="hs81")}
        xs8 = acp.tile([P, KX * 2, 16], f8, tag="xs8", name="xs8")
        for l in (0, 1):
            nc.vector.memset(
                hs8[l][:, :, :].rearrange("p a b -> p (a b)"), 0.0)

        def r3(v):  # [P, K] -> [P, K, 1]
            return v[:, :].rearrange("p (k o) -> p k o", o=1)

        def make_split(v32, K, s8, pref):
            """v32 [P,K] f32 -> s8 [P,K,16] f8: slot0 = e4m3(v*64),
            slot1 = e4m3(v*64 - slot0)"""
            sa = acp.tile([P, K], f32, tag=pref + "a", name=pref + "a")
            sb = acp.tile([P, K], f32, tag=pref + "b", name=pref + "b")
            nc.scalar.activation(out=s8[:, :, 0:1], in_=r3(v32), func=COPY,
                                 scale=SV)
            nc.scalar.activation(out=sa[:, :], in_=v32[:, :], func=COPY,
                                 scale=SV)
            nc.vector.tensor_copy(out=r3(sb), in_=s8[:, :, 0:1])
            nc.vector.tensor_tensor(out=sa[:, :], in0=sa[:, :], in1=sb[:, :],
                                    op=OP.subtract)
            nc.vector.tensor_copy(out=s8[:, :, 1:2], in_=r3(sa))

        # x for step 0
        xT = acp.tile([P, KX * 2], f32, tag="xT", name="xT")
        nc.scalar.dma_start(out=xT[:, :], in_=xt0d[:, :])
        make_split(xT, KX * 2, xs8, "xs")

        hT = {}
        for l in (0, 1):
            hT[l] = acp.tile([P, KH * 2], f32, tag=f"hT{l}", name=f"hT{l}")

        def lhsT_of(layer, kk):
            if layer == 0:
                if kk < KX:
                    return xs8[:, 2 * kk:2 * kk + 2, 0:2]
                k = kk - KX
                return hs8[0][:, 2 * k:2 * k + 2, 0:2]
            if kk < KH:
                return hs8[1][:, 2 * kk:2 * kk + 2, 0:2]
            k = kk - KH
            return hs8[0][:, 2 * k:2 * k + 2, 0:2]

        def mm_chunk(ps, layer, kk, wap, start, stop):
            w3 = wap.rearrange("p (i n) -> p i n", i=2)
            la = lhsT_of(layer, kk)
            for s in range(NSEG):
                nc.tensor.matmul(
                    ps[0:2, s * 512:(s + 1) * 512], lhsT=la,
                    rhs=w3[:, :, s * 512:(s + 1) * 512],
                    start=start, stop=stop, perf_mode=DR)

        def stream_dma(out, in_):
            nc.sync.dma_start(out=out, in_=in_)

        def layer_mms(layer, gbase, nch, order=None):
            """emit hi + lo matmuls for one layer, interleaved per chunk so
            the TE queue never waits on a later-dependency chunk."""
            ps = psp.tile([2, Gc], f32, tag="ps", name="ps")
            kks = list(order) if order is not None else list(range(nch))
            n_mm = nch * 2 + sum(1 for kk in kks if gbase + kk >= NRES)
            done = 0
            for kk in kks:
                g = gbase + kk
                if g < NRES:
                    done += 1
                    mm_chunk(ps, layer, kk, wres_ap(g),
                             start=(done == 1), stop=(done == n_mm))
                else:
                    wt = wsp.tile([P, CHB], f8, tag="wst", name="wst")
                    stream_dma(wt[:, :],
                               whim[:, (g - NRES) * CHB:(g - NRES + 1) * CHB])
                    done += 1
                    mm_chunk(ps, layer, kk, wt[:, :], start=(done == 1),
                             stop=(done == n_mm))
                wt = wsp.tile([P, CHB], f8, tag="wst", name="wst")
                stream_dma(wt[:, :], wlod[:, g * CHB:(g + 1) * CHB])
                done += 1
                mm_chunk(ps, layer, kk, wt[:, :], start=False,
                         stop=(done == n_mm))
            return ps

        def g4(v):  # [P, 16] -> [P, 4, 4]: [:, l, b]
            return v[:, :].rearrange("p (l b) -> p l b", b=4)

        g2 = gap.tile([2, Gc], f32, tag="g2", name="g2")

        def layer_tail(ps, l):
            """psum [2, 2048] -> gates 2D -> activations -> c/h update.
            Returns h slice [P, 4] f32 (local h, l = p*4 + j)."""
            nc.scalar.activation(out=g2[:, :], in_=ps[0:2, :], func=COPY)
            ga = gap.tile([P, 16], f32, tag=f"ga{l}", name=f"ga{l}")
            gb = gap.tile([P, 16], f32, tag=f"gb{l}", name=f"gb{l}")
            nc.scalar.dma_start(out=ga[:, :], in_=g2[0:1, :])
            nc.gpsimd.dma_start(out=gb[:, :], in_=g2[1:2, :])
            nc.vector.tensor_tensor(out=ga[:, :], in0=ga[:, :], in1=gb[:, :],
                                    op=OP.add)
            nc.vector.tensor_tensor(out=ga[:, :], in0=ga[:, :],
                                    in1=bsb[l][:, :], op=OP.add)
            gv = g4(ga)
            for b, fn in enumerate((SIG, SIG, TANH, SIG)):
                nc.scalar.activation(out=gv[:, :, b:b + 1],
                                     in_=gv[:, :, b:b + 1], func=fn,
                                     scale=SINV)
            i_g, f_g = gv[:, :, 0:1], gv[:, :, 1:2]
            g_g, o_g = gv[:, :, 2:3], gv[:, :, 3:4]
            c = c_t[l]
            cv = c[:, :].rearrange("p (l o) -> p l o", o=1)
            tmp = gap.tile([P, 4], f32, tag=f"tm{l}", name=f"tm{l}")
            tv = tmp[:, :].rearrange("p (l o) -> p l o", o=1)
            nc.vector.tensor_tensor(out=cv, in0=cv, in1=f_g, op=OP.mult)
            nc.vector.tensor_tensor(out=tv, in0=i_g, in1=g_g, op=OP.mult)
            nc.vector.tensor_tensor(out=cv, in0=cv, in1=tv, op=OP.add)
            nc.scalar.activation(out=tv, in_=cv, func=TANH)
            hsb = gap.tile([P, 4], f32, tag=f"h{l}sb", name=f"h{l}sb")
            hv = hsb[:, :].rearrange("p (l o) -> p l o", o=1)
            nc.vector.tensor_tensor(out=hv, in0=o_g, in1=tv, op=OP.mult)
            return hsb

        def all_gather(hsb, l):
            agin = drp.tile([1, Hc], f32, tag=f"agi{l}", name=f"agi{l}")
            nc.scalar.dma_start(out=agin[:, :], in_=hsb[:, :])
            agout = drp.tile([1, H], f32, tag=f"ago{l}", name=f"ago{l}")
            nc.gpsimd.collective_compute(
                "AllGather", mybir.AluOpType.bypass,
                replica_groups=[list(range(NCORES))],
                ins=[agin[:, :].opt()], outs=[agout[:, :].opt()])
            nc.gpsimd.dma_start(out=hT[l][:, :], in_=agout[:, :])
            make_split(hT[l], KH * 2, hs8[l], f"hs{l}")

        # small argmax tiles
        pm = stp.tile([P, 8], f32, tag="pm", name="pm")
        pidx = stp.tile([P, 8], u32, tag="pidx", name="pidx")
        gm = stp.tile([P, 1], f32, tag="gm", name="gm")
        ism = stp.tile([P, 1], f32, tag="ism", name="ism")
        cand = stp.tile([P, 1], f32, tag="cand", name="cand")
        tokm = stp.tile([P, 1], f32, tag="tokm", name="tokm")
        toku = stp.tile([P, 1], u32, tag="toku", name="toku")
        off = stp.tile([P, 1], u32, tag="off", name="off")

        # layer-0 chunk order: h0 chunks first (ready at step start), x
        # chunks last (wait on prev step's argmax/embed fetch)
        L0_ORDER = list(range(KX, NCH0)) + list(range(KX))

        for t in range(T):
            # ---------------- layer 0 ----------------
            ps = layer_mms(0, 0, NCH0, order=L0_ORDER)
            h0sb = layer_tail(ps, 0)
            all_gather(h0sb, 0)

            # ---------------- layer 1 ----------------
            ps = layer_mms(1, NCH0, NCH1)
            h1sb = layer_tail(ps, 1)
            nc.scalar.dma_start(out=outd.ap()[t:t + 1, :], in_=h1sb[:, :])
            if t == T - 1:
                break
            all_gather(h1sb, 1)

            # ---- greedy argmax over full h1 (2D layout) + embed fetch ----
            nc.vector.max(out=pm[:, :], in_=hT[1][:, :])
            nc.vector.max_index(out=pidx[:, :], in_max=pm[:, :],
                                in_values=hT[1][:, :])
            nc.gpsimd.partition_all_reduce(gm[:, :], pm[:, 0:1], channels=P,
                                           reduce_op=bass_isa.ReduceOp.max)
            nc.vector.tensor_tensor(out=ism[:, :], in0=pm[:, 0:1],
                                    in1=gm[:, :], op=OP.is_equal)
            nc.vector.tensor_copy(out=cand[:, :], in_=pidx[:, 0:1])
            nc.vector.tensor_tensor(out=cand[:, :], in0=cand[:, :],
                                    in1=p32[:, :], op=OP.add)
            # cand = (4096 - gidx) * is_max ; reduce max -> 4096 - argmin idx
            nc.vector.tensor_scalar(out=cand[:, :], in0=cand[:, :],
                                    scalar1=-1.0, scalar2=4096.0,
                                    op0=OP.mult, op1=OP.add)
            nc.vector.tensor_tensor(out=cand[:, :], in0=cand[:, :],
                                    in1=ism[:, :], op=OP.mult)
            nc.gpsimd.partition_all_reduce(tokm[:, :], cand[:, :], channels=P,
                                           reduce_op=bass_isa.ReduceOp.max)
            nc.vector.tensor_scalar(out=tokm[:, :], in0=tokm[:, :],
                                    scalar1=-1.0, scalar2=4096.0,
                                    op0=OP.mult, op1=OP.add)
            nc.vector.tensor_copy(out=toku[:, :], in_=tokm[:, :])
            nc.vector.tensor_scalar(out=off[:, :], in0=toku[:, :],
                                    scalar1=P, scalar2=None, op0=OP.mult)
            nc.vector.tensor_tensor(out=off[:, :], in0=off[:, :],
                                    in1=iot[:, :], op=OP.add)
            xT = acp.tile([P, KX * 2], f32, tag="xT", name="xT")
            nc.gpsimd.indirect_dma_start(
                out=xT[:, :], out_offset=None, in_=embtd[:, :],
                in_offset=bass.IndirectOffsetOnAxis(ap=off[:, :], axis=0))
            make_split(xT, KX * 2, xs8, "xs")

    nc.compile()
    return nc


# --------------------------------------------------------------------------
# host-side data prep
# --------------------------------------------------------------------------
E4NP = ml_dtypes.float8_e4m3fn if hasattr(ml_dtypes, "float8_e4m3fn") \
    else ml_dtypes.float8_e4m3


def _e4(v):
    return np.clip(v, -240.0, 240.0).astype(E4NP)


def _chunk_cols(layer):
    """per chunk kk: int array idx[P, 2] of contraction column indices"""
    p = np.arange(P)
    cols = []
    if layer == 0:
        for kk in range(KX):
            cols.append(np.stack([p * (2 * KX) + 2 * kk,
                                  p * (2 * KX) + 2 * kk + 1], 1))
        for kk in range(KH):
            cols.append(np.stack([E + p * (2 * KH) + 2 * kk,
                                  E + p * (2 * KH) + 2 * kk + 1], 1))
    else:
        for kk in range(KH):  # h1 part (first H columns)
            cols.append(np.stack([p * (2 * KH) + 2 * kk,
                                  p * (2 * KH) + 2 * kk + 1], 1))
        for kk in range(KH):  # h0 part
            cols.append(np.stack([H + p * (2 * KH) + 2 * kk,
                                  H + p * (2 * KH) + 2 * kk + 1], 1))
    return cols


def _pack_chunks(Wq, cols):
    """Wq (Gc, K) fp8; returns [P, nch*CHB] uint8-layout fp8 array"""
    out = np.empty((P, len(cols) * CHB), E4NP)
    for g, idx in enumerate(cols):
        # chunk [P, 2, Gc]: [p, i, n] = Wq[n, idx[p, i]]
        ch = Wq[:, idx]                    # (Gc, P, 2)
        out[:, g * CHB:(g + 1) * CHB] = \
            ch.transpose(1, 2, 0).reshape(P, CHB)
    return out


def prep_inputs(inputs):
    fv = np.asarray(inputs["feature_vector"], np.float32)
    embed = np.asarray(inputs["embed"], np.float32)
    W0 = np.concatenate([np.asarray(inputs["W_ih0"], np.float32),
                         np.asarray(inputs["W_hh0"], np.float32)], axis=1)
    W1 = np.concatenate([np.asarray(inputs["W_hh1"], np.float32),
                         np.asarray(inputs["W_ih1"], np.float32)], axis=1)
    b0 = np.asarray(inputs["b_ih0"], np.float32) + np.asarray(
        inputs["b_hh0"], np.float32)
    b1 = np.asarray(inputs["b_ih1"], np.float32) + np.asarray(
        inputs["b_hh1"], np.float32)

    cols0 = _chunk_cols(0)
    cols1 = _chunk_cols(1)

    embt = np.ascontiguousarray(embed.reshape(H * P, 4))
    xt0 = np.ascontiguousarray(fv.reshape(P, 4))
    shared = {
        "embtd": embt,
        "xt0d": xt0,
        "iotd": np.arange(P, dtype=np.uint32).reshape(P, 1),
        "i32d": (np.arange(P, dtype=np.float32) * (2 * KH)).reshape(P, 1),
    }

    n = np.arange(Gc)
    in_maps = []
    for c in range(NCORES):
        rows = (n % 4) * H + c * Hc + (n // 4)   # psum col n = 4*l + gate
        hi_parts, lo_parts = [], []
        for W, cols in ((W0[rows], cols0), (W1[rows], cols1)):
            Ws = W * np.float32(SW)
            hi = _e4(Ws)
            lo = _e4(Ws - hi.astype(np.float32))
            hi_parts.append(_pack_chunks(hi, cols))
            lo_parts.append(_pack_chunks(lo, cols))
        hi_all = np.concatenate(hi_parts, 1)
        lo_all = np.concatenate(lo_parts, 1)
        in_maps.append(dict(
            shared,
            whir=np.ascontiguousarray(hi_all[:, :NRES * CHB]),
            whim=np.ascontiguousarray(hi_all[:, NRES * CHB:]),
            wlod=np.ascontiguousarray(lo_all),
            b0d=np.ascontiguousarray(
                (b0[rows] * np.float32(SW * SV)).reshape(P, 16)),
            b1d=np.ascontiguousarray(
                (b1[rows] * np.float32(SW * SV)).reshape(P, 16)),
        ))
    return in_maps


_NC_CACHE = {}


def _get_nc():
    if "nc" not in _NC_CACHE:
        _NC_CACHE["nc"] = build_nc()
    return _NC_CACHE["nc"]


def run(inputs, trace=False):
    from concourse.bass_utils import run_bass_kernel_spmd
    nc = _get_nc()
    in_maps = prep_inputs(inputs)
    res = run_bass_kernel_spmd(nc, in_maps, core_ids=list(range(NCORES)),
                               trace=trace)
    full = np.empty((T, H), np.float32)
    for c in range(NCORES):
        full[:, c * Hc:(c + 1) * Hc] = res.results[c]["out"]
    return full, res


def kernel(**inputs):
    full, _ = run(inputs, trace=False)
    return full


# revision 43
# speedup vs baseline: 1.0385x; 1.0385x over previous
"""2-layer LSTM greedy decoder (H=4096, E=512, 15 steps) on 8 trn2 NeuronCores.

Tensor-parallel over the 4*H gate dimension, core c owning h indices
[c*512, (c+1)*512).  Weights quantized as e4m3 hi + e4m3 lo planes at scale
512 (8.6 effective bits; validated to preserve the greedy argmax trajectory
with rel err 4.3e-3).  The hi plane is SBUF-resident (loaded once); the lo
plane streams from HBM every step (1 B/weight).  All gate matvecs run as
fp8 DoubleRow matmuls (256-deep contraction, 2x column rate).

Activations split on device into e4m3 hi+lo pairs at scale 64 (M=2
stationary columns -> psum rows folded after).  Gates live in a 2D
[128,16] per-partition layout (psum column n = 4*l + gate) so bias add,
sigmoid/tanh, c/h updates and argmax all run partition-parallel.
"""

import numpy as np
import ml_dtypes

H = 4096
E = 512
T = 15
NCORES = 8
P = 128

Hc = H // NCORES          # 512 h elements per core
Gc = 4 * Hc               # 2048 gate columns per core
KH = H // 256             # 16 DoubleRow chunks per h vector
KX = E // 256             # 2 chunks for x
NCH0 = KX + KH            # 18 layer-0 chunks (x first, then h0)
NCH1 = 2 * KH             # 32 layer-1 chunks (h1 first, then h0)
NCH = NCH0 + NCH1         # 50 total
N_MV = 5                  # trailing hi chunks streamed (SBUF headroom)
NRES = NCH - N_MV
NSEG = 4                  # psum 512-column segments
CHB = 2 * Gc              # fp8 bytes per chunk per partition (4096)

SW = 512.0                # weight scale
SV = 64.0                 # activation scale
SINV = 1.0 / (SW * SV)    # 2^-15, applied inside sigmoid/tanh


def build_nc():
    import concourse.bass as bass
    import concourse.mybir as mybir
    import concourse.tile as tile
    from concourse import bacc, bass_isa

    dt = mybir.dt
    AF = mybir.ActivationFunctionType
    OP = mybir.AluOpType
    DR = mybir.MatmulPerfMode.DoubleRow
    f32, f16, f8, u32 = dt.float32, dt.float16, dt.float8e4, dt.uint32
    SIG, TANH, COPY = AF.Sigmoid, AF.Tanh, AF.Copy

    nc = bacc.Bacc("TRN2", target_bir_lowering=False, debug=False,
                   num_devices=NCORES)

    whir = nc.dram_tensor("whir", [P, NRES * CHB], f8, kind="ExternalInput")
    whim = nc.dram_tensor("whim", [P, N_MV * CHB], f8, kind="ExternalInput")
    wlod = nc.dram_tensor("wlod", [P, NCH * CHB], f8, kind="ExternalInput")
    b0d = nc.dram_tensor("b0d", [P, 16], f32, kind="ExternalInput")
    b1d = nc.dram_tensor("b1d", [P, 16], f32, kind="ExternalInput")
    embtd = nc.dram_tensor("embtd", [H * P, 4], f32, kind="ExternalInput")
    xt0d = nc.dram_tensor("xt0d", [P, 4], f32, kind="ExternalInput")
    iotd = nc.dram_tensor("iotd", [P, 1], u32, kind="ExternalInput")
    i32d = nc.dram_tensor("i32d", [P, 1], f32, kind="ExternalInput")
    outd = nc.dram_tensor("out", [T, Hc], f32, kind="ExternalOutput")

    with tile.TileContext(nc) as tc, \
            tc.tile_pool(name="wres", bufs=1) as wrp, \
            tc.tile_pool(name="wst", bufs=4) as wsp, \
            tc.tile_pool(name="act", bufs=1) as acp, \
            tc.tile_pool(name="gat", bufs=1) as gap, \
            tc.tile_pool(name="st", bufs=1) as stp, \
            tc.tile_pool(name="psum", bufs=2, space="PSUM") as psp, \
            tc.tile_pool(name="dram", bufs=2, space="DRAM") as drp:

        # ---- constants / persistent state ----
        # resident hi plane, split per layer so step-0 layer 0 starts early
        wres0 = wrp.tile([P, NCH0 * CHB], f8, tag="wres0", name="wres0")
        nc.sync.dma_start(out=wres0[:, :], in_=whir[:, :NCH0 * CHB])
        NR1 = NRES - NCH0
        wres1 = wrp.tile([P, NR1 * CHB], f8, tag="wres1", name="wres1")
        NLOAD = 4
        bnd = [NCH0 * CHB + round(i * NR1 / NLOAD) * CHB
               for i in range(NLOAD + 1)]
        for i in range(NLOAD):
            nc.sync.dma_start(
                out=wres1[:, bnd[i] - NCH0 * CHB:bnd[i + 1] - NCH0 * CHB],
                in_=whir[:, bnd[i]:bnd[i + 1]])

        def wres_ap(g):
            if g < NCH0:
                return wres0[:, g * CHB:(g + 1) * CHB]
            return wres1[:, (g - NCH0) * CHB:(g - NCH0 + 1) * CHB]

        bsb = {}
        for l, bd in ((0, b0d), (1, b1d)):
            bsb[l] = stp.tile([P, 16], f32, tag=f"b{l}sb", name=f"b{l}sb")
            nc.scalar.dma_start(out=bsb[l][:, :], in_=bd[:, :])
        iot = stp.tile([P, 1], u32, tag="iot", name="iot")
        nc.scalar.dma_start(out=iot[:, :], in_=iotd[:, :])
        p32 = stp.tile([P, 1], f32, tag="p32", name="p32")
        nc.scalar.dma_start(out=p32[:, :], in_=i32d[:, :])
        c_t = {}
        for l in (0, 1):
            c_t[l] = stp.tile([P, 4], f32, tag=f"c{l}", name=f"c{l}")
            nc.vector.memset(c_t[l][:, :], 0.0)

        # fp8 hi/lo splits of activations (slot0=hi, slot1=lo, 16B cells)
        hs8 = {0: acp.tile([P, KH * 2, 16], f8, tag="hs80", name="hs80"),
               1: acp.tile([P, KH * 2, 16], f8, tag="hs81", name="hs81")}
        xs8 = acp.tile([P, KX * 2, 16], f8, tag="xs8", name="xs8")
        for l in (0, 1):
            nc.vector.memset(
                hs8[l][:, :, :].rearrange("p a b -> p (a b)"), 0.0)

        def r3(v):  # [P, K] -> [P, K, 1]
            return v[:, :].rearrange("p (k o) -> p k o", o=1)

        def make_split(v32, K, s8, pref):
            """v32 [P,K] f32 -> s8 [P,K,16] f8: slot0 = e4m3(v*64),
            slot1 = e4m3(v*64 - slot0)"""
            sa = acp.tile([P, K], f32, tag=pref + "a", name=pref + "a")
            sb = acp.tile([P, K], f32, tag=pref + "b", name=pref + "b")
            nc.scalar.activation(out=s8[:, :, 0:1], in_=r3(v32), func=COPY,
                                 scale=SV)
            nc.scalar.activation(out=sa[:, :], in_=v32[:, :], func=COPY,
                                 scale=SV)
            nc.vector.tensor_copy(out=r3(sb), in_=s8[:, :, 0:1])
            nc.vector.tensor_tensor(out=sa[:, :], in0=sa[:, :], in1=sb[:, :],
                                    op=OP.subtract)
            nc.vector.tensor_copy(out=s8[:, :, 1:2], in_=r3(sa))

        # x for step 0
        xT = acp.tile([P, KX * 2], f32, tag="xT", name="xT")
        nc.scalar.dma_start(out=xT[:, :], in_=xt0d[:, :])
        make_split(xT, KX * 2, xs8, "xs")

        hT = {}
        for l in (0, 1):
            hT[l] = acp.tile([P, KH * 2], f32, tag=f"hT{l}", name=f"hT{l}")

        def lhsT_of(layer, kk):
            if layer == 0:
                if kk < KX:
                    return xs8[:, 2 * kk:2 * kk + 2, 0:2]
                k = kk - KX
                return hs8[0][:, 2 * k:2 * k + 2, 0:2]
            if kk < KH:
                return hs8[1][:, 2 * kk:2 * kk + 2, 0:2]
            k = kk - KH
            return hs8[0][:, 2 * k:2 * k + 2, 0:2]

        def mm_chunk(ps, layer, kk, wap, start, stop):
            w3 = wap.rearrange("p (i n) -> p i n", i=2)
            la = lhsT_of(layer, kk)
            for s in range(NSEG):
                nc.tensor.matmul(
                    ps[0:2, s * 512:(s + 1) * 512], lhsT=la,
                    rhs=w3[:, :, s * 512:(s + 1) * 512],
                    start=start, stop=stop, perf_mode=DR)

        def stream_dma(out, in_):
            nc.sync.dma_start(out=out, in_=in_)

        def layer_mms(layer, gbase, nch, order=None):
            """emit hi + lo matmuls for one layer, interleaved per chunk so
            the TE queue never waits on a later-dependency chunk."""
            ps = psp.tile([2, Gc], f32, tag="ps", name="ps")
            kks = list(order) if order is not None else list(range(nch))
            n_mm = nch * 2 + sum(1 for kk in kks if gbase + kk >= NRES)
            done = 0
            for kk in kks:
                g = gbase + kk
                if g < NRES:
                    done += 1
                    mm_chunk(ps, layer, kk, wres_ap(g),
                             start=(done == 1), stop=(done == n_mm))
                else:
                    wt = wsp.tile([P, CHB], f8, tag="wst", name="wst")
                    stream_dma(wt[:, :],
                               whim[:, (g - NRES) * CHB:(g - NRES + 1) * CHB])
                    done += 1
                    mm_chunk(ps, layer, kk, wt[:, :], start=(done == 1),
                             stop=(done == n_mm))
                wt = wsp.tile([P, CHB], f8, tag="wst", name="wst")
                stream_dma(wt[:, :], wlod[:, g * CHB:(g + 1) * CHB])
                done += 1
                mm_chunk(ps, layer, kk, wt[:, :], start=False,
                         stop=(done == n_mm))
            return ps

        def g4(v):  # [P, 16] -> [P, 4, 4]: [:, l, b]
            return v[:, :].rearrange("p (l b) -> p l b", b=4)

        g2 = gap.tile([2, Gc], f32, tag="g2", name="g2")

        def layer_tail(ps, l):
            """psum [2, 2048] -> gates 2D -> activations -> c/h update.
            Returns h slice [P, 4] f32 (local h, l = p*4 + j)."""
            nc.scalar.activation(out=g2[:, :], in_=ps[0:2, :], func=COPY)
            ga = gap.tile([P, 16], f32, tag=f"ga{l}", name=f"ga{l}")
            gb = gap.tile([P, 16], f32, tag=f"gb{l}", name=f"gb{l}")
            nc.scalar.dma_start(out=ga[:, :], in_=g2[0:1, :])
            nc.gpsimd.dma_start(out=gb[:, :], in_=g2[1:2, :])
            nc.vector.tensor_tensor(out=ga[:, :], in0=ga[:, :], in1=gb[:, :],
                                    op=OP.add)
            nc.vector.tensor_tensor(out=ga[:, :], in0=ga[:, :],
                                    in1=bsb[l][:, :], op=OP.add)
            gv = g4(ga)
            for b, fn in enumerate((SIG, SIG, TANH, SIG)):
                nc.scalar.activation(out=gv[:, :, b:b + 1],
                                     in_=gv[:, :, b:b + 1], func=fn,
                                     scale=SINV)
            i_g, f_g = gv[:, :, 0:1], gv[:, :, 1:2]
            g_g, o_g = gv[:, :, 2:3], gv[:, :, 3:4]
            c = c_t[l]
            cv = c[:, :].rearrange("p (l o) -> p l o", o=1)
            tmp = gap.tile([P, 4], f32, tag=f"tm{l}", name=f"tm{l}")
            tv = tmp[:, :].rearrange("p (l o) -> p l o", o=1)
            nc.vector.tensor_tensor(out=cv, in0=cv, in1=f_g, op=OP.mult)
            nc.vector.tensor_tensor(out=tv, in0=i_g, in1=g_g, op=OP.mult)
            nc.vector.tensor_tensor(out=cv, in0=cv, in1=tv, op=OP.add)
            nc.scalar.activation(out=tv, in_=cv, func=TANH)
            hsb = gap.tile([P, 4], f32, tag=f"h{l}sb", name=f"h{l}sb")
            hv = hsb[:, :].rearrange("p (l o) -> p l o", o=1)
            nc.vector.tensor_tensor(out=hv, in0=o_g, in1=tv, op=OP.mult)
            return hsb

        def all_gather(hsb, l):
            agin = drp.tile([1, Hc], f32, tag=f"agi{l}", name=f"agi{l}")
            nc.scalar.dma_start(out=agin[:, :], in_=hsb[:, :])
            agout = drp.tile([1, H], f32, tag=f"ago{l}", name=f"ago{l}")
            nc.gpsimd.collective_compute(
                "AllGather", mybir.AluOpType.bypass,
                replica_groups=[list(range(NCORES))],
                ins=[agin[:, :].opt()], outs=[agout[:, :].opt()])
            nc.gpsimd.dma_start(out=hT[l][:, :], in_=agout[:, :])
            make_split(hT[l], KH * 2, hs8[l], f"hs{l}")

        # small argmax tiles
        pm = stp.tile([P, 8], f32, tag="pm", name="pm")
        pidx = stp.tile([P, 8], u32, tag="pidx", name="pidx")
        gm = stp.tile([P, 1], f32, tag="gm", name="gm")
        ism = stp.tile([P, 1], f32, tag="ism", name="ism")
        cand = stp.tile([P, 1], f32, tag="cand", name="cand")
        tokm = stp.tile([P, 1], f32, tag="tokm", name="tokm")
        toku = stp.tile([P, 1], u32, tag="toku", name="toku")
        off = stp.tile([P, 1], u32, tag="off", name="off")

        # layer-0 chunk order: h0 chunks first (ready at step start), x
        # chunks last (wait on prev step's argmax/embed fetch)
        L0_ORDER = list(range(KX, NCH0)) + list(range(KX))

        for t in range(T):
            # ---------------- layer 0 ----------------
            ps = layer_mms(0, 0, NCH0, order=L0_ORDER)
            h0sb = layer_tail(ps, 0)
            all_gather(h0sb, 0)

            # ---------------- layer 1 ----------------
            ps = layer_mms(1, NCH0, NCH1)
            h1sb = layer_tail(ps, 1)
            nc.scalar.dma_start(out=outd.ap()[t:t + 1, :], in_=h1sb[:, :])
            if t == T - 1:
                break
            all_gather(h1sb, 1)

            # ---- greedy argmax over full h1 (2D layout) + embed fetch ----
            nc.vector.max(out=pm[:, :], in_=hT[1][:, :])
            nc.vector.max_index(out=pidx[:, :], in_max=pm[:, :],
                                in_values=hT[1][:, :])
            nc.gpsimd.partition_all_reduce(gm[:, :], pm[:, 0:1], channels=P,
                                           reduce_op=bass_isa.ReduceOp.max)
            nc.vector.tensor_tensor(out=ism[:, :], in0=pm[:, 0:1],
                                    in1=gm[:, :], op=OP.is_equal)
            nc.vector.tensor_copy(out=cand[:, :], in_=pidx[:, 0:1])
            nc.vector.tensor_tensor(out=cand[:, :], in0=cand[:, :],
                                    in1=p32[:, :], op=OP.add)
            # cand = (4096 - gidx) * is_max ; reduce max -> 4096 - argmin idx
            nc.vector.tensor_scalar(out=cand[:, :], in0=cand[:, :],
                                    scalar1=-1.0, scalar2=4096.0,
                                    op0=OP.mult, op1=OP.add)
            nc.vector.tensor_tensor(out=cand[:, :], in0=cand[:, :],
                                    in1=ism[:, :], op=OP.mult)
            nc.gpsimd.partition_all_reduce(tokm[:, :], cand[:, :], channels=P,
                                           reduce_op=bass_isa.ReduceOp.max)
            nc.vector.tensor_scalar(out=tokm[:, :], in0=tokm[:, :],
                                    scalar1=-1.0, scalar2=4096.0,
                                    op0=OP.mult, op1=OP.add)
            nc.vector.tensor_copy(out=toku[:, :], in_=tokm[:, :])
            nc.vector.tensor_scalar(out=off[:, :], in0=toku[:, :],
                                    scalar1=P, scalar2=None, op0=OP.mult)
            nc.vector.tensor_tensor(out=off[:, :], in0=off[:, :],
                                    in1=iot[:, :], op=OP.add)
            xT = acp.tile([P, KX * 2], f32, tag="xT", name="xT")
            nc.gpsimd.indirect_dma_start(
                out=xT[:, :], out_offset=None, in_=embtd[:, :],
                in_offset=bass.IndirectOffsetOnAxis(ap=off[:, :], axis=0))
            make_split(xT, KX * 2, xs8, "xs")

    nc.compile()
    return nc


# --------------------------------------------------------------------------
# host-side data prep
# --------------------------------------------------------------------------
E4NP = ml_dtypes.float8_e4m3fn if hasattr(ml_dtypes, "float8_e4m3fn") \
    else ml_dtypes.float8_e4m3


def _e4(v):
    return np.clip(v, -240.0, 240.0).astype(E4NP)


def _chunk_cols(layer):
    """per chunk kk: int array idx[P, 2] of contraction column indices"""
    p = np.arange(P)
    cols = []
    if layer == 0:
        for kk in range(KX):
            cols.append(np.stack([p * (2 * KX) + 2 * kk,
                                  p * (2 * KX) + 2 * kk + 1], 1))
        for kk in range(KH):
            cols.append(np.stack([E + p * (2 * KH) + 2 * kk,
                                  E + p * (2 * KH) + 2 * kk + 1], 1))
    else:
        for kk in range(KH):  # h1 part (first H columns)
            cols.append(np.stack([p * (2 * KH) + 2 * kk,
                                  p * (2 * KH) + 2 * kk + 1], 1))
        for kk in range(KH):  # h0 part
            cols.append(np.stack([H + p * (2 * KH) + 2 * kk,
                                  H + p * (2 * KH) + 2 * kk + 1], 1))
    return cols


def _pack_chunks(Wq, cols):
    """Wq (Gc, K) fp8; returns [P, nch*CHB] uint8-layout fp8 array"""
    out = np.empty((P, len(cols) * CHB), E4NP)
    for g, idx in enumerate(cols):
        # chunk [P, 2, Gc]: [p, i, n] = Wq[n, idx[p, i]]
        ch = Wq[:, idx]                    # (Gc, P, 2)
        out[:, g * CHB:(g + 1) * CHB] = \
            ch.transpose(1, 2, 0).reshape(P, CHB)
    return out


def prep_inputs(inputs):
    fv = np.asarray(inputs["feature_vector"], np.float32)
    embed = np.asarray(inputs["embed"], np.float32)
    W0 = np.concatenate([np.asarray(inputs["W_ih0"], np.float32),
                         np.asarray(inputs["W_hh0"], np.float32)], axis=1)
    W1 = np.concatenate([np.asarray(inputs["W_hh1"], np.float32),
                         np.asarray(inputs["W_ih1"], np.float32)], axis=1)
    b0 = np.asarray(inputs["b_ih0"], np.float32) + np.asarray(
        inputs["b_hh0"], np.float32)
    b1 = np.asarray(inputs["b_ih1"], np.float32) + np.asarray(
        inputs["b_hh1"], np.float32)

    cols0 = _chunk_cols(0)
    cols1 = _chunk_cols(1)

    embt = np.ascontiguousarray(embed.reshape(H * P, 4))
    xt0 = np.ascontiguousarray(fv.reshape(P, 4))
    shared = {
        "embtd": embt,
        "xt0d": xt0,
        "iotd": np.arange(P, dtype=np.uint32).reshape(P, 1),
        "i32d": (np.arange(P, dtype=np.float32) * (2 * KH)).reshape(P, 1),
    }

    n = np.arange(Gc)
    in_maps = []
    for c in range(NCORES):
        rows = (n % 4) * H + c * Hc + (n // 4)   # psum col n = 4*l + gate
        hi_parts, lo_parts = [], []
        for W, cols in ((W0[rows], cols0), (W1[rows], cols1)):
            Ws = W * np.float32(SW)
            hi = _e4(Ws)
            lo = _e4(Ws - hi.astype(np.float32))
            hi_parts.append(_pack_chunks(hi, cols))
            lo_parts.append(_pack_chunks(lo, cols))
        hi_all = np.concatenate(hi_parts, 1)
        lo_all = np.concatenate(lo_parts, 1)
        in_maps.append(dict(
            shared,
            whir=np.ascontiguousarray(hi_all[:, :NRES * CHB]),
            whim=np.ascontiguousarray(hi_all[:, NRES * CHB:]),
            wlod=np.ascontiguousarray(lo_all),
            b0d=np.ascontiguousarray(
                (b0[rows] * np.float32(SW * SV)).reshape(P, 16)),
            b1d=np.ascontiguousarray(
                (b1[rows] * np.float32(SW * SV)).reshape(P, 16)),
        ))
    return in_maps


_NC_CACHE = {}


def _get_nc():
    if "nc" not in _NC_CACHE:
        _NC_CACHE["nc"] = build_nc()
    return _NC_CACHE["nc"]


def run(inputs, trace=False):
    from concourse.bass_utils import run_bass_kernel_spmd
    nc = _get_nc()
    in_maps = prep_inputs(inputs)
    res = run_bass_kernel_spmd(nc, in_maps, core_ids=list(range(NCORES)),
                               trace=trace)
    full = np.empty((T, H), np.float32)
    for c in range(NCORES):
        full[:, c * Hc:(c + 1) * Hc] = res.results[c]["out"]
    return full, res


def kernel(**inputs):
    full, _ = run(inputs, trace=False)
    return full
